# revision 3
# baseline (speedup 1.0000x reference)
"""Causal dilated 1D conv (KW=4, dilation=8) as shifted matmuls on 8 TRN2 cores.

out[b,o,t] = sum_{k,c} W[o, c*4+k] * x[b, c, t + k*8 - 24]

Sharding: data-parallel over batch (16 batches -> 2 per core). Each core runs
an identical program: weights stationary in SBUF, x streamed in 2048-wide time
blocks (+24 halo), 16 accumulating matmuls (4 c-chunks x 4 taps) per
(out-chunk, 512-wide mm-block) PSUM group. Matmuls run in float32r (fp32
data, FP22 multiply) which streams at 1 cycle/row for free-dim >= 256.

Weight tiles are split per (oc, cc, tap) and DMA'd in consumption order
after the first x tiles so the PE starts matmuls ~1.5us in and the HAM
clock warms immediately.
"""

import numpy as np

B = 16
C_IN = 512
C_OUT = 512
T = 8192
KW = 4
DIL = 8
PAD = (KW - 1) * DIL  # 24

N_CORES = 8
B_PER = B // N_CORES  # 2
P = 128
MMBLK = 512           # matmul free dim / PSUM bank
XBLK = 2048           # x-tile time width (4 mm-blocks)
NXT = T // XBLK       # 4 x-tiles per batch row
NMM = XBLK // MMBLK   # 4 mm-blocks per x-tile
NCC = C_IN // P       # 4
NOC = C_OUT // P      # 4

_cache = {}


def _build():
    import concourse.tile as tile
    from concourse import bacc, mybir

    nc = bacc.Bacc("TRN2", target_bir_lowering=False, debug=False,
                   num_devices=N_CORES)
    x = nc.dram_tensor("x", [B_PER, C_IN, T + PAD], mybir.dt.float32r,
                       kind="ExternalInput").ap()
    # weights pre-arranged on host as [oc, cc, tap, c=128, o=128]
    wt = nc.dram_tensor("wt", [NOC, NCC, KW, P, P], mybir.dt.float32r,
                        kind="ExternalInput").ap()
    out = nc.dram_tensor("out", [B_PER, C_OUT, T], mybir.dt.float32,
                         kind="ExternalOutput").ap()
    f32 = mybir.dt.float32
    f32r = mybir.dt.float32r

    with tile.TileContext(nc) as tc:
        with tc.tile_pool(name="wpool", bufs=1) as wpool, \
             tc.tile_pool(name="xpool", bufs=3) as xpool, \
             tc.tile_pool(name="opool", bufs=6) as opool, \
             tc.tile_pool(name="pspool", bufs=8, space="PSUM") as pspool:

            # First x tiles (b=0, xb=0) before any weights so the PE can
            # start as soon as the first weight tile lands.
            first_xts = []
            for cc in range(NCC):
                xt = xpool.tile([P, XBLK + PAD], f32r,
                                name=f"xt{cc}", tag=f"xt{cc}")
                nc.sync.dma_start(xt[:], x[0, cc * P:(cc + 1) * P, 0:XBLK + PAD])
                first_xts.append(xt)

            # Weights resident for the whole kernel, DMA'd in the order the
            # first oc-sweep consumes them: oc outer, cc, tap inner.
            wtiles = [[[None] * KW for _ in range(NCC)] for _ in range(NOC)]
            for oc in range(NOC):
                for cc in range(NCC):
                    for k in range(KW):
                        wtile = wpool.tile([P, P], f32r,
                                           name=f"w_{oc}_{cc}_{k}",
                                           tag=f"w_{oc}_{cc}_{k}")
                        nc.sync.dma_start(wtile[:], wt[oc, cc, k])
                        wtiles[oc][cc][k] = wtile

            for b in range(B_PER):
                for xb in range(NXT):
                    if b == 0 and xb == 0:
                        xts = first_xts
                    else:
                        xts = []
                        for cc in range(NCC):
                            xt = xpool.tile([P, XBLK + PAD], f32r,
                                            name=f"xt{cc}", tag=f"xt{cc}")
                            nc.sync.dma_start(
                                xt[:],
                                x[b, cc * P:(cc + 1) * P,
                                  xb * XBLK: xb * XBLK + XBLK + PAD])
                            xts.append(xt)
                    for mb in range(NMM):
                        toff = mb * MMBLK
                        for oc in range(NOC):
                            ps = pspool.tile([P, MMBLK], f32, name="ps",
                                             tag="ps")
                            idx = 0
                            for cc in range(NCC):
                                for k in range(KW):
                                    nc.tensor.matmul(
                                        ps[:],
                                        wtiles[oc][cc][k][:],
                                        xts[cc][:, toff + k * DIL:
                                                toff + k * DIL + MMBLK],
                                        start=(idx == 0),
                                        stop=(idx == NCC * KW - 1),
                                    )
                                    idx += 1
                            ot = opool.tile([P, MMBLK], f32, name="ot",
                                            tag="ot")
                            nc.vector.tensor_copy(ot[:], ps[:])
                            nc.sync.dma_start(
                                out[b, oc * P:(oc + 1) * P,
                                    xb * XBLK + toff:
                                    xb * XBLK + toff + MMBLK],
                                ot[:])

    nc.compile()
    return nc


def _get_nc():
    if "nc" not in _cache:
        _cache["nc"] = _build()
    return _cache["nc"]


def _make_in_maps(x, W):
    xpad = np.pad(np.ascontiguousarray(x, dtype=np.float32),
                  ((0, 0), (0, 0), (PAD, 0)))
    w = np.ascontiguousarray(W, dtype=np.float32).reshape(C_OUT, C_IN, KW)
    # wt[oc, cc, k, c, o] = W[oc*128+o, (cc*128+c)*KW + k]
    wt = np.transpose(
        w.reshape(NOC, P, NCC, P, KW), (0, 2, 4, 3, 1)
    ).copy()  # [NOC, NCC, KW, c=128, o=128]
    return [{"x": np.ascontiguousarray(xpad[i * B_PER:(i + 1) * B_PER]),
             "wt": wt} for i in range(N_CORES)]


def kernel(x, W):
    from concourse.bass_utils import run_bass_kernel_spmd

    nc = _get_nc()
    in_maps = _make_in_maps(x, W)
    res = run_bass_kernel_spmd(nc, in_maps, list(range(N_CORES)))
    return np.concatenate([r["out"] for r in res.results], axis=0)


# revision 4
# speedup vs baseline: 1.0512x; 1.0512x over previous
"""Causal dilated 1D conv (KW=4, dilation=8) as shifted matmuls on 8 TRN2 cores.

out[b,o,t] = sum_{k,c} W[o, c*4+k] * x[b, c, t + k*8 - 24]

Sharding: data-parallel over batch (16 batches -> 2 per core). Each core runs
an identical program: weights stationary in SBUF, x streamed in 512-wide time
blocks (+24 halo), 16 accumulating matmuls (4 c-chunks x 4 taps) per
(out-chunk, time-block) PSUM group. Matmuls run in float32r (fp32 data,
FP22 multiply) which streams at 1 cycle/row for free-dim >= 256.

DMA order at startup: first time-block's x tiles (1.1MB) land before the
16 weight tiles (4MB, issued in first-group consumption order), so the PE
starts matmuls ~3.5us in and stays weight-arrival-fed, warming the HAM
clock immediately.
"""

import numpy as np

B = 16
C_IN = 512
C_OUT = 512
T = 8192
KW = 4
DIL = 8
PAD = (KW - 1) * DIL  # 24

N_CORES = 8
B_PER = B // N_CORES  # 2
P = 128
TBLK = 512
NT = T // TBLK        # 16
NCC = C_IN // P       # 4
NOC = C_OUT // P      # 4

_cache = {}


def _build():
    import concourse.tile as tile
    from concourse import bacc, mybir

    nc = bacc.Bacc("TRN2", target_bir_lowering=False, debug=False,
                   num_devices=N_CORES)
    x = nc.dram_tensor("x", [B_PER, C_IN, T + PAD], mybir.dt.float32r,
                       kind="ExternalInput").ap()
    # weights pre-arranged on host as [cc, tap, c=128, o=512]
    wt = nc.dram_tensor("wt", [NCC, KW, P, C_OUT], mybir.dt.float32r,
                        kind="ExternalInput").ap()
    out = nc.dram_tensor("out", [B_PER, C_OUT, T], mybir.dt.float32,
                         kind="ExternalOutput").ap()
    f32 = mybir.dt.float32
    f32r = mybir.dt.float32r

    with tile.TileContext(nc) as tc:
        with tc.tile_pool(name="wpool", bufs=1) as wpool, \
             tc.tile_pool(name="xpool", bufs=3) as xpool, \
             tc.tile_pool(name="opool", bufs=6) as opool, \
             tc.tile_pool(name="pspool", bufs=8, space="PSUM") as pspool:

            # First time-block's x tiles before any weights: small (1.1MB)
            # so the first weight tile lands early and the PE starts fast.
            first_xts = []
            for cc in range(NCC):
                xt = xpool.tile([P, TBLK + PAD], f32r,
                                name=f"xt{cc}", tag=f"xt{cc}")
                nc.sync.dma_start(xt[:], x[0, cc * P:(cc + 1) * P,
                                           0:TBLK + PAD])
                first_xts.append(xt)

            # Weights resident for the whole kernel: [c=128, o=512] per
            # (c-chunk, tap), issued in the order the first group consumes
            # them (cc outer, tap inner).
            wtiles = [[None] * KW for _ in range(NCC)]
            for cc in range(NCC):
                for k in range(KW):
                    wtile = wpool.tile([P, C_OUT], f32r, name=f"w_{cc}_{k}",
                                       tag=f"w_{cc}_{k}")
                    nc.sync.dma_start(wtile[:], wt[cc, k])
                    wtiles[cc][k] = wtile

            for b in range(B_PER):
                for tb in range(NT):
                    if b == 0 and tb == 0:
                        xts = first_xts
                    else:
                        xts = []
                        for cc in range(NCC):
                            xt = xpool.tile([P, TBLK + PAD], f32r,
                                            name=f"xt{cc}", tag=f"xt{cc}")
                            nc.sync.dma_start(
                                xt[:],
                                x[b, cc * P:(cc + 1) * P,
                                  tb * TBLK: tb * TBLK + TBLK + PAD])
                            xts.append(xt)
                    for oc in range(NOC):
                        ps = pspool.tile([P, TBLK], f32, name="ps", tag="ps")
                        idx = 0
                        for cc in range(NCC):
                            for k in range(KW):
                                nc.tensor.matmul(
                                    ps[:],
                                    wtiles[cc][k][:, oc * P:(oc + 1) * P],
                                    xts[cc][:, k * DIL: k * DIL + TBLK],
                                    start=(idx == 0),
                                    stop=(idx == NCC * KW - 1),
                                )
                                idx += 1
                        ot = opool.tile([P, TBLK], f32, name="ot", tag="ot")
                        nc.vector.tensor_copy(ot[:], ps[:])
                        nc.sync.dma_start(
                            out[b, oc * P:(oc + 1) * P,
                                tb * TBLK:(tb + 1) * TBLK],
                            ot[:])

    nc.compile()
    return nc


def _get_nc():
    if "nc" not in _cache:
        _cache["nc"] = _build()
    return _cache["nc"]


def _make_in_maps(x, W):
    xpad = np.pad(np.ascontiguousarray(x, dtype=np.float32),
                  ((0, 0), (0, 0), (PAD, 0)))
    w = np.ascontiguousarray(W, dtype=np.float32).reshape(C_OUT, C_IN, KW)
    # wt[cc, k, c, o] = W[o, (cc*128+c)*KW + k]
    wt = np.transpose(w.reshape(C_OUT, NCC, P, KW), (1, 3, 2, 0)).copy()
    return [{"x": np.ascontiguousarray(xpad[i * B_PER:(i + 1) * B_PER]),
             "wt": wt} for i in range(N_CORES)]


def kernel(x, W):
    from concourse.bass_utils import run_bass_kernel_spmd

    nc = _get_nc()
    in_maps = _make_in_maps(x, W)
    res = run_bass_kernel_spmd(nc, in_maps, list(range(N_CORES)))
    return np.concatenate([r["out"] for r in res.results], axis=0)
